# revision 13
# baseline (speedup 1.0000x reference)
"""DCRNN kernel for Trainium2 — 8-core SPMD, chunked GRU, windowed layer 2.

Sharding: node dim split 8 ways (256 nodes/core).  Each core receives a
cyclically ROTATED copy of the graph (A, x127 rotated by k*256 - W) so one
SPMD program serves all cores: its GRU window = local nodes [0, W+256) =
global [k*256-W, (k+1)*256).  Layer 1 aggregation is replicated (each core
computes full H1 — cheaper than a collective); layer 2 aggregation, GI and
the GRU run only on the local window.  Core 0's first W warmup columns are
garbage (wrap) — a per-core mask input resets chain 0 on core 0 exactly at
the warmup/real boundary (the true h=0 start of the global chain).
"""

import numpy as np
import ml_dtypes
from contextlib import ExitStack

import concourse.bass as bass
import concourse.tile as tile
from concourse import bacc, mybir
from concourse import bass_utils
from concourse.bass_interp import get_hw_module

N, T, F_IN, H, OUT = 2048, 128, 64, 128, 1
NCORE = 8
NLOC = N // NCORE          # 256 real nodes per core
NCHUNK = N // 128          # 16 src chunks for aggregation
L = 16                     # GRU steps per chunk
W = 16                     # warmup steps
C = NLOC // L              # 16 parallel chains per core
S = W + L                  # 32 batched steps
WIN = W + NLOC             # 272 window columns (warmup + real)
FP = mybir.dt.float32
BF = mybir.dt.float16
AF = mybir.ActivationFunctionType
OP = mybir.AluOpType
BF_NP = np.float16

_CACHE = {}


def _build():
    nc = bacc.Bacc("TRN2", target_bir_lowering=False, debug=False,
                   enable_asserts=False, num_devices=1)

    xT_ap = nc.dram_tensor("xT", [F_IN, N], BF, kind="ExternalInput").ap()
    aT_ap = nc.dram_tensor("aT", [N, N], BF, kind="ExternalInput").ap()
    w1_ap = nc.dram_tensor("w1", [F_IN, H], BF, kind="ExternalInput").ap()
    w2_ap = nc.dram_tensor("w2", [H, H], BF, kind="ExternalInput").ap()
    b1_ap = nc.dram_tensor("b1c", [H, 1], FP, kind="ExternalInput").ap()
    b2_ap = nc.dram_tensor("b2c", [H, 1], FP, kind="ExternalInput").ap()
    wihT_ap = nc.dram_tensor("wihT", [H, 3 * H], BF, kind="ExternalInput").ap()
    uT_ap = nc.dram_tensor("uT", [H, 3 * H], FP, kind="ExternalInput").ap()
    bsum_ap = nc.dram_tensor("bsum", [H, 3], FP, kind="ExternalInput").ap()
    bnr_ap = nc.dram_tensor("bnr", [H, 1], FP, kind="ExternalInput").ap()
    fcT_ap = nc.dram_tensor("fcT", [H, OUT], FP, kind="ExternalInput").ap()
    id_ap = nc.dram_tensor("ident", [128, 128], FP, kind="ExternalInput").ap()
    mask_ap = nc.dram_tensor("mask", [128, C], FP, kind="ExternalInput").ap()
    y_ap = nc.dram_tensor("y", [OUT, NLOC], FP, kind="ExternalOutput").ap()

    with tile.TileContext(nc) as tc:
        with ExitStack() as ctx:
            const = ctx.enter_context(tc.tile_pool(name="const", bufs=1))
            xT_sb = const.tile([F_IN, N], BF)
            w1_sb = const.tile([F_IN, H], BF)
            w2_sb = const.tile([H, H], BF)
            b1_sb = const.tile([H, 1], FP)
            b2_sb = const.tile([H, 1], FP)
            wihT_sb = const.tile([H, 3 * H], BF)
            uT_sb = const.tile([H, 3 * H], FP)
            bsum_sb = const.tile([H, 3], FP)
            bnr_sb = const.tile([H, 1], FP)
            fcT_sb = const.tile([H, OUT], FP)
            id_sb = const.tile([128, 128], FP)
            mask_sb = const.tile([128, C], FP)
            for sb, ap in [(xT_sb, xT_ap), (w1_sb, w1_ap), (w2_sb, w2_ap),
                           (b1_sb, b1_ap), (b2_sb, b2_ap), (wihT_sb, wihT_ap),
                           (uT_sb, uT_ap), (bsum_sb, bsum_ap), (bnr_sb, bnr_ap),
                           (fcT_sb, fcT_ap), (id_sb, id_ap), (mask_sb, mask_ap)]:
                nc.sync.dma_start(sb[:], ap[:])

            a_res = ctx.enter_context(tc.tile_pool(name="a_res", bufs=1))
            a_chunks = []
            for c in range(NCHUNK):
                a_c = a_res.tile([128, N], BF, name=f"a_c{c}")
                nc.sync.dma_start(a_c[:], aT_ap[c * 128:(c + 1) * 128, :])
                a_chunks.append(a_c[:])

            big = ctx.enter_context(tc.tile_pool(name="big", bufs=1))
            haggT_sb = big.tile([128, N], BF)    # full H1, feat-major
            x2T_sb = big.tile([128, WIN], BF)    # windowed X2, feat-major
            gir_sb = big.tile([128, WIN], FP)
            giz_sb = big.tile([128, WIN], FP)
            gin_sb = big.tile([128, WIN], FP)
            y_sb = big.tile([OUT, NLOC], FP)
            warm_sb = big.tile([128, 1], FP)
            nc.vector.memset(warm_sb[:], 0.0)
            nc.scalar.activation(warm_sb[:], warm_sb[:], AF.Sigmoid)

            # ---- layer 1: full linear + full aggregation (replicated) ----
            with ExitStack() as c1:
                lin_pool = c1.enter_context(tc.tile_pool(name="lin1", bufs=2, space="PSUM"))
                h1 = c1.enter_context(tc.tile_pool(name="h1sb", bufs=1))
                h_sb = h1.tile([128, N], BF)
                for c in range(NCHUNK):
                    ps = lin_pool.tile([128, H], FP)
                    nc.tensor.matmul(ps[:], xT_sb[:, c * 128:(c + 1) * 128],
                                     w1_sb[:], start=True, stop=True)
                    nc.vector.tensor_copy(h_sb[:, c * 128:(c + 1) * 128], ps[:])
                agg_pool = c1.enter_context(tc.tile_pool(name="agg1", bufs=1, space="PSUM"))
                ps_agg = [agg_pool.tile([128, 512], FP, name=f"agg1{q}") for q in range(4)]
                for c in range(NCHUNK):
                    for q in range(4):
                        nc.tensor.matmul(
                            ps_agg[q][:], h_sb[:, c * 128:(c + 1) * 128],
                            a_chunks[c][:, q * 512:(q + 1) * 512],
                            start=(c == 0), stop=(c == NCHUNK - 1))
                for q in range(4):
                    nc.scalar.activation(haggT_sb[:, q * 512:(q + 1) * 512],
                                         ps_agg[q][:], AF.Identity, bias=b1_sb[:])

            # ---- layer 2: full linear, WINDOWED aggregation ----
            with ExitStack() as c2:
                lin_pool = c2.enter_context(tc.tile_pool(name="lin2", bufs=2, space="PSUM"))
                h2p = c2.enter_context(tc.tile_pool(name="h2sb", bufs=1))
                h2_sb = h2p.tile([128, N], BF)
                for c in range(NCHUNK):
                    ps = lin_pool.tile([128, H], FP)
                    nc.tensor.matmul(ps[:], haggT_sb[:, c * 128:(c + 1) * 128],
                                     w2_sb[:], start=True, stop=True)
                    nc.vector.tensor_copy(h2_sb[:, c * 128:(c + 1) * 128], ps[:])
                agg_pool = c2.enter_context(tc.tile_pool(name="agg2", bufs=1, space="PSUM"))
                ps_agg = agg_pool.tile([128, WIN], FP)
                for c in range(NCHUNK):
                    nc.tensor.matmul(ps_agg[:], h2_sb[:, c * 128:(c + 1) * 128],
                                     a_chunks[c][:, 0:WIN],
                                     start=(c == 0), stop=(c == NCHUNK - 1))
                nc.scalar.activation(x2T_sb[:], ps_agg[:], AF.Identity,
                                     bias=b2_sb[:])

            # ---- GI over the window ----
            with ExitStack() as c3:
                gi_ps = c3.enter_context(tc.tile_pool(name="gi_ps", bufs=3, space="PSUM"))
                for g, buf in enumerate([gir_sb, giz_sb, gin_sb]):
                    ps = gi_ps.tile([128, WIN], FP)
                    nc.tensor.matmul(ps[:], wihT_sb[:, g * 128:(g + 1) * 128],
                                     x2T_sb[:], start=True, stop=True)
                    nc.scalar.activation(buf[:], ps[:], AF.Identity,
                                         bias=bsum_sb[:, g:g + 1])

            gir_v = gir_sb[:].rearrange("p (c l) -> p l c", l=L)
            giz_v = giz_sb[:].rearrange("p (c l) -> p l c", l=L)
            gin_v = gin_sb[:].rearrange("p (c l) -> p l c", l=L)
            y_v = y_sb[:].rearrange("p (c l) -> p l c", l=L)

            def gi_view(v, s):
                return v[:, s % L, s // L: s // L + C]

            # ---- batched GRU: 32 steps over 16 chains ----
            with ExitStack() as c4:
                rz_pool = c4.enter_context(tc.tile_pool(name="ps_rz", bufs=2, space="PSUM"))
                n_pool = c4.enter_context(tc.tile_pool(name="ps_n", bufs=2, space="PSUM"))
                y_pool = c4.enter_context(tc.tile_pool(name="ps_y", bufs=2, space="PSUM"))
                gates = c4.enter_context(tc.tile_pool(name="gates", bufs=3))
                state = c4.enter_context(tc.tile_pool(name="state", bufs=3))
                u_r = uT_sb[:, 0:128]
                u_z = uT_sb[:, 128:256]
                u_n = uT_sb[:, 256:384]

                hf = state.tile([128, C], FP, name="hf_init")
                nc.vector.memset(hf[:], 0.0)

                for s in range(S):
                    ps_r = rz_pool.tile([128, C], FP, tag="psr")
                    ps_z = rz_pool.tile([128, C], FP, tag="psz")
                    ps_n = n_pool.tile([128, C], FP)
                    nc.tensor.matmul(ps_r[:], id_sb[:], gi_view(gir_v, s),
                                     start=True, stop=False)
                    nc.tensor.matmul(ps_z[:], id_sb[:], gi_view(giz_v, s),
                                     start=True, stop=False)
                    nc.tensor.matmul(ps_r[:], u_r, hf[:], start=False, stop=True)
                    nc.tensor.matmul(ps_z[:], u_z, hf[:], start=False, stop=True)
                    nc.tensor.matmul(ps_n[:], u_n, hf[:], start=True, stop=True)

                    rz_sb = gates.tile([128, 2 * C], FP)
                    nc.scalar.activation(rz_sb[:, 0:C], ps_r[:], AF.Sigmoid)
                    nc.scalar.activation(rz_sb[:, C:2 * C], ps_z[:], AF.Sigmoid)
                    t2 = gates.tile([128, C], FP)
                    nc.vector.scalar_tensor_tensor(
                        t2[:], ps_n[:], bnr_sb[:], rz_sb[:, 0:C],
                        op0=OP.add, op1=OP.mult)
                    t3 = gates.tile([128, C], FP)
                    nc.vector.tensor_add(t3[:], t2[:], gi_view(gin_v, s))
                    n_sb = gates.tile([128, C], FP)
                    nc.scalar.activation(n_sb[:], t3[:], AF.Tanh)

                    d_sb = gates.tile([128, C], FP)
                    nc.vector.tensor_sub(d_sb[:], hf[:], n_sb[:])
                    dz_sb = gates.tile([128, C], FP)
                    nc.vector.tensor_mul(dz_sb[:], d_sb[:], rz_sb[:, C:2 * C])
                    hf_new = state.tile([128, C], FP)
                    if s == W - 1:
                        # mask: zero chain-0 state on core 0 only (exact h0)
                        t4 = gates.tile([128, C], FP)
                        nc.vector.tensor_add(t4[:], dz_sb[:], n_sb[:])
                        nc.vector.tensor_mul(hf_new[:], t4[:], mask_sb[:])
                    else:
                        nc.vector.tensor_add(hf_new[:], dz_sb[:], n_sb[:])
                    if s >= W:
                        j = s - W
                        ps_y = y_pool.tile([OUT, C], FP)
                        nc.tensor.matmul(ps_y[:], fcT_sb[:], hf_new[:],
                                         start=True, stop=True)
                        nc.scalar.activation(y_v[:, j, 0:C], ps_y[:], AF.Copy)
                    hf = hf_new

            nc.sync.dma_start(y_ap[:], y_sb[:])

    nc.compile()
    nc.m_pre_hw = nc.m
    nc.m = get_hw_module(nc.m)
    return nc


def _host_prep(x, edge_index, W1, b1, W2, b2, w_ih, w_hh, b_ih, b_hh, fc_w, fc_b):
    x127 = np.asarray(x[:, T - 1, :], dtype=np.float32)          # [N, F_IN]
    src = np.asarray(edge_index[0], dtype=np.int64)
    dst = np.asarray(edge_index[1], dtype=np.int64)
    deg = np.bincount(dst, minlength=N).astype(np.float64) + 1.0
    dinv = deg ** -0.5
    aT = np.zeros((N, N), dtype=np.float32)
    np.add.at(aT, (src, dst), (dinv[src] * dinv[dst]).astype(np.float32))
    aT[np.arange(N), np.arange(N)] += (dinv * dinv).astype(np.float32)

    b_hh64 = np.asarray(b_hh, dtype=np.float64)
    b_ih64 = np.asarray(b_ih, dtype=np.float64)
    bsum = np.concatenate([
        b_ih64[0:H] + b_hh64[0:H],
        b_ih64[H:2 * H] + b_hh64[H:2 * H],
        b_ih64[2 * H:3 * H],
    ]).astype(np.float32)

    shared = {
        "w1": np.asarray(W1, dtype=np.float32).astype(BF_NP),
        "w2": np.asarray(W2, dtype=np.float32).astype(BF_NP),
        "b1c": np.ascontiguousarray(np.asarray(b1, np.float32).reshape(H, 1)),
        "b2c": np.ascontiguousarray(np.asarray(b2, np.float32).reshape(H, 1)),
        "wihT": np.ascontiguousarray(np.asarray(w_ih, np.float32).T).astype(BF_NP),
        "uT": np.ascontiguousarray(np.asarray(w_hh, np.float32).T),
        "bsum": np.ascontiguousarray(bsum.reshape(3, H).T),
        "bnr": np.ascontiguousarray(
            b_hh64[2 * H:3 * H].astype(np.float32).reshape(H, 1)),
        "fcT": np.ascontiguousarray(np.asarray(fc_w, np.float32).T),
        "ident": np.eye(128, dtype=np.float32),
    }
    aT_bf = aT.astype(BF_NP)
    xT_bf = np.ascontiguousarray(x127.T).astype(BF_NP)
    in_maps = []
    for k in range(NCORE):
        perm = (np.arange(N) + (k * NLOC - W)) % N
        mask = np.ones((128, C), dtype=np.float32)
        if k == 0:
            mask[:, 0] = 0.0
        m = dict(shared)
        m["aT"] = np.ascontiguousarray(aT_bf[perm][:, perm])
        m["xT"] = np.ascontiguousarray(xT_bf[:, perm])
        m["mask"] = mask
        in_maps.append(m)
    return in_maps


def kernel(**inputs):
    inputs.pop("_debug", None)
    inputs.pop("_trace", None)
    if "main" not in _CACHE:
        _CACHE["main"] = _build()
    nc = _CACHE["main"]
    in_maps = _host_prep(**inputs)
    res = bass_utils.run_bass_kernel_spmd(nc, in_maps, core_ids=list(range(NCORE)))
    fc_b = np.asarray(inputs["fc_b"], dtype=np.float32)
    y = np.concatenate([r["y"].reshape(OUT, NLOC).T for r in res.results], axis=0)
    return (y + fc_b[None, :]).astype(np.float32)
